# revision 20
# baseline (speedup 1.0000x reference)
"""Pairwise KL divergence kernel for Trainium2, SPMD across 8 NeuronCores.

out[n, m] = sum_d a[n,d]*(log a[n,d] - log b[m,d])
          = ent[n] - (a @ log(b)^T)[n, m],  ent = rowsum(a * log a)

Sharding: a (and output rows) split 8 ways; b replicated.
Per core: a_shard (1024, 64), b (8192, 64) -> out_shard (1024, 8192).

Structure (v5):
  - output written as bf16 (16 MB/core vs 32 MB) and converted to fp32 on
    the host; bf16 rounding adds ~2e-3 rel err, well under the 2e-2 gate.
  - b processed in 4 pairs of 1024-row chunks. Each pair loads as one
    linear DMA (2 KB lines). Transposes are full [128,128] PE ops: input
    is the j-th row-slice of BOTH chunks -> output partitions 0-63 hold
    chunk-lo's d, 64-127 chunk-hi's d. Halves transpose + Ln-evac count.
  - aT lives on partitions 0-63; one SBUF->SBUF DMA replicates it to
    partitions 64-127 so lo/hi matmuls both find their stationary operand
    at the right base partition.
  - per (pair, n-tile): 4 fp32r matmuls -> 2 psum tiles; evacuation fused
    with the entropy bias, split 3 ways across ACT/DVE/GPSIMD; one
    2048-col bf16 DMA (4 KB lines) alternating the SP/ACT HW DGE queues.
"""

import numpy as np

N, M, D = 8192, 8192, 64
NCORES = 8
NSHARD = N // NCORES          # 1024 rows of a per core
NT = NSHARD // 128            # 8 n-tiles per core
PAIR = 2048                   # b rows per chunk-pair
NPAIR = M // PAIR             # 4 pairs
CT = 8                        # j-values per chunk (rows per partition)

# matmul operand dtype: "fp32" (4 cyc/row), "fp32r" (1 cyc/row), "bf16"
MM_DTYPE = "fp32r"
# output HBM dtype: "bf16" halves the output write traffic
OUT_DTYPE = "bf16"
# evacuation engines: gpsimd cannot access PSUM on TRN2, so evac is split
# ACT:DVE = 7:9 (ACT also carries the Ln evacuations + la)
EVAC_POOL = False

_CACHE = {}


def _build(mm_dtype, out_dtype, evac_pool):
    from contextlib import ExitStack

    import concourse.bacc as bacc_mod
    import concourse.bass as bass
    import concourse.mybir as mybir
    import concourse.tile as tile
    from concourse.masks import make_identity

    FP32 = mybir.dt.float32
    AF = mybir.ActivationFunctionType
    ALU = mybir.AluOpType
    AX = mybir.AxisListType

    ODT = mybir.dt.bfloat16 if out_dtype == "bf16" else FP32
    MMDT = {
        "fp32r": mybir.dt.float32r,
        "bf16": mybir.dt.bfloat16,
        "fp32": FP32,
    }[mm_dtype]

    nc = bacc_mod.Bacc()
    a_d = nc.dram_tensor("a", [NSHARD, D], FP32, kind="ExternalInput")
    b_d = nc.dram_tensor("b", [M, D], FP32, kind="ExternalInput")
    out_d = nc.dram_tensor("out", [NSHARD, M], ODT, kind="ExternalOutput")

    with tile.TileContext(nc) as tc, ExitStack() as ctx:
        consts = ctx.enter_context(tc.tile_pool(name="consts", bufs=1))
        apool = ctx.enter_context(tc.tile_pool(name="apool", bufs=1))
        bpool = ctx.enter_context(tc.tile_pool(name="bpool", bufs=2))
        lbtp = ctx.enter_context(tc.tile_pool(name="lbtp", bufs=2))
        tpsum = ctx.enter_context(tc.tile_pool(name="tpsum", bufs=2, space="PSUM"))
        mmps = ctx.enter_context(tc.tile_pool(name="mmps", bufs=3, space="PSUM"))
        stage = ctx.enter_context(tc.tile_pool(name="stage", bufs=6))

        ident = consts.tile([128, 128], FP32)
        make_identity(nc, ident)
        # Dummy transpose so PE observes the gpsimd (ident) sem here: the
        # matmul/LDW struct only carries ONE sync wait, so later transposes
        # must each need at most one sem (codegen: "Too many sync waits").
        warm = tpsum.tile([128, 128], FP32, tag="tp")
        nc.tensor.transpose(warm, ident, ident)

        # Pair loads: row m = H*2048 + half*1024 + p*8 + j sits at
        # [p, half, j, :] — per (partition, half) one contiguous 2 KB line.
        b_r = b_d[:, :].rearrange(
            "(H half p j) d -> p H half (j d)", H=NPAIR, half=2, p=128
        )

        def load_pair(H):
            t = bpool.tile([128, 2, CT, D], FP32, tag="b_nat")
            nc.sync.dma_start(
                out=t, in_=b_r[:, H, :, :].rearrange("p half (j d) -> p half j d", d=D)
            )
            return t

        # b pair 0 in flight before the a prologue issues its DMA.
        b_tiles = [load_pair(0), None]

        # ---------------- a prologue ----------------
        # a loads on the ACT HW DGE queue so it transfers CONCURRENTLY with
        # b pair 0 on the SP queue.
        a_nat = apool.tile([128, NT, D], FP32)        # row t*128+p at [p, t, :]
        nc.scalar.dma_start(out=a_nat, in_=a_d[:, :].rearrange("(t p) d -> p t d", p=128))
        la = apool.tile([128, NT, D], FP32)
        nc.scalar.activation(la, a_nat, AF.Ln)
        prod = apool.tile([128, NT, D], FP32)
        nc.vector.tensor_mul(prod, a_nat, la)
        ent = apool.tile([128, NT], FP32)
        for t in range(NT):
            nc.vector.reduce_sum(ent[:, t : t + 1], prod[:, t, :], axis=AX.X)

        aT = apool.tile([64, NT, 128], MMDT)          # aT[:, t, :] = a tile t transposed
        for g in range(2):
            tp = tpsum.tile([64, 4, 128], FP32, tag="tp")
            for j in range(4):
                nc.tensor.transpose(tp[:, j], a_nat[:, g * 4 + j, :], ident)
            # DVE evacuation keeps ACT free for Ln work in the head
            nc.vector.tensor_copy(aT[:, g * 4 : (g + 1) * 4, :], tp)

        # evac engine schedule: ACT also does Ln-evacs + la, so DVE takes a
        # slightly larger share (9 of 16).
        def evac_engine(i):
            r = i % 16
            if r in (0, 2, 5, 7, 9, 11, 14):
                return nc.scalar
            return nc.vector

        # ---------------- pair-pipelined main loop ----------------
        ev = 0
        for H in range(NPAIR):
            if H + 1 < NPAIR:
                b_tiles[(H + 1) % 2] = load_pair(H + 1)
            b_nat = b_tiles[H % 2]
            # transpose raw b per chunk half, fuse Ln into the PSUM->SBUF
            # evacuation. Column p of transpose j of half c holds row
            # H*2048 + c*1024 + 8p + j, so the evac writes lbT columns {8p+j}
            # (stride-8 view) and the GEMM reads m-contiguous 512-wide slices.
            lbTs = []
            for half in range(2):
                lbT = lbtp.tile([64, CT * 128], MMDT, tag=f"lbT{half}")
                lbT_v = lbT[:, :].rearrange("d (p j) -> d j p", j=CT)
                for g in range(CT // 4):
                    tp = tpsum.tile([64, 4, 128], FP32, tag="tp")
                    for j in range(4):
                        nc.tensor.transpose(
                            tp[:, j], b_nat[:, half, g * 4 + j, :], ident
                        )
                    nc.scalar.activation(lbT_v[:, g * 4 : (g + 1) * 4, :], tp, AF.Ln)
                lbTs.append(lbT)
            for t in range(NT):
                ent_t = ent[:, t : t + 1]
                out_sb = stage.tile([128, 2, 2, 512], ODT, tag="out_sb")
                for half in range(2):
                    ps = mmps.tile([128, 2, 512], FP32, tag="ps")
                    for j in range(2):
                        nc.tensor.matmul(
                            ps[:, j],
                            aT[:, t, :],
                            lbTs[half][:, j * 512 : (j + 1) * 512],
                            start=True,
                            stop=True,
                        )
                    eng = evac_engine(ev)
                    ev += 1
                    if eng is nc.scalar:
                        nc.scalar.activation(
                            out_sb[:, half], ps, AF.Identity, bias=ent_t, scale=-1.0
                        )
                    else:
                        eng.tensor_scalar(
                            out_sb[:, half], ps, -1.0, ent_t, ALU.mult, ALU.add
                        )
                # one 2048-col DMA per (pair, n-tile): 4 KB lines, alternating
                # the two HW DGE queues so queue re-arm bubbles overlap.
                dma_eng = nc.sync if (H * NT + t) % 2 == 0 else nc.scalar
                dma_eng.dma_start(
                    out=out_d[
                        t * 128 : (t + 1) * 128, H * PAIR : (H + 1) * PAIR
                    ].rearrange("p (c m) -> p c m", m=512),
                    in_=out_sb[:, :, :, :].rearrange("p a b m -> p (a b) m"),
                )
    # bacc lowering: splits multi-sem waits onto event-semaphore/nop
    # instructions (HW allows one sync wait per engine instruction).
    nc.compile()
    return nc


def _run(a, b, trace=False):
    from concourse.bass_utils import run_bass_kernel_spmd

    key = (MM_DTYPE, OUT_DTYPE, EVAC_POOL)
    if key not in _CACHE:
        _CACHE[key] = _build(*key)
    nc = _CACHE[key]
    a = np.ascontiguousarray(np.asarray(a, dtype=np.float32))
    b = np.ascontiguousarray(np.asarray(b, dtype=np.float32))
    in_maps = [
        {"a": a[i * NSHARD : (i + 1) * NSHARD], "b": b} for i in range(NCORES)
    ]
    res = run_bass_kernel_spmd(nc, in_maps, list(range(NCORES)), trace=trace)
    out = np.concatenate(
        [np.asarray(r["out"], dtype=np.float32) for r in res.results], axis=0
    )
    return out, res


def kernel(a, b):
    out, _ = _run(a, b, trace=False)
    return out


# revision 21
# speedup vs baseline: 1.3451x; 1.3451x over previous
"""Pairwise KL divergence kernel for Trainium2, SPMD across 8 NeuronCores.

out[n, m] = sum_d a[n,d]*(log a[n,d] - log b[m,d])
          = ent[n] - (a @ log(b)^T)[n, m],  ent = rowsum(a * log a)

Sharding: a (and output rows) split 8 ways; b replicated.
Per core: a_shard (1024, 64), b (8192, 64) -> out_shard (1024, 8192).

Structure:
  - output written as bf16 (16 MB/core vs 32 MB) and converted to fp32 on
    the host; bf16 rounding adds ~2e-3 rel err, well under the 2e-2 gate.
  - b loaded in 8 linear 1024-row chunks (2 KB lines); chunk transposes on
    PE with Ln fused into the ACT PSUM->SBUF evacuation (strided write so
    the GEMM reads m-contiguous 512-wide f32r slices).
  - chunk h+1's transposes are interleaved INTO chunk h's n-tile loop so
    the evacuation engines never idle at chunk boundaries.
  - per (chunk, n-tile): 2 fp32r matmuls -> psum (128, 2, 512); evac fused
    with the entropy bias, alternating ACT/DVE 7:9; one 2 KB-line bf16 DMA
    per n-tile alternating the SP/ACT HW DGE queues (single-dependency
    DMAs only — multi-sem waits serialize the queue).
"""

import numpy as np

N, M, D = 8192, 8192, 64
NCORES = 8
NSHARD = N // NCORES          # 1024 rows of a per core
NT = NSHARD // 128            # 8 n-tiles per core
HCH = 1024                    # b rows per chunk
NCH = M // HCH                # 8 chunks
CT = HCH // 128               # 8 b row-tiles per chunk

# matmul operand dtype: "fp32" (4 cyc/row), "fp32r" (1 cyc/row), "bf16"
MM_DTYPE = "fp32r"
# output HBM dtype: "bf16" halves the output write traffic
OUT_DTYPE = "bf16"

_CACHE = {}


def _build(mm_dtype, out_dtype):
    from contextlib import ExitStack

    import concourse.bacc as bacc_mod
    import concourse.bass as bass
    import concourse.mybir as mybir
    import concourse.tile as tile
    from concourse.masks import make_identity

    FP32 = mybir.dt.float32
    AF = mybir.ActivationFunctionType
    ALU = mybir.AluOpType
    AX = mybir.AxisListType

    ODT = mybir.dt.bfloat16 if out_dtype == "bf16" else FP32
    MMDT = {
        "fp32r": mybir.dt.float32r,
        "bf16": mybir.dt.bfloat16,
        "fp32": FP32,
    }[mm_dtype]

    nc = bacc_mod.Bacc()
    a_d = nc.dram_tensor("a", [NSHARD, D], FP32, kind="ExternalInput")
    b_d = nc.dram_tensor("b", [M, D], FP32, kind="ExternalInput")
    out_d = nc.dram_tensor("out", [NSHARD, M], ODT, kind="ExternalOutput")

    with tile.TileContext(nc) as tc, ExitStack() as ctx:
        consts = ctx.enter_context(tc.tile_pool(name="consts", bufs=1))
        apool = ctx.enter_context(tc.tile_pool(name="apool", bufs=1))
        bpool = ctx.enter_context(tc.tile_pool(name="bpool", bufs=2))
        lbtp = ctx.enter_context(tc.tile_pool(name="lbtp", bufs=2))
        tpsum = ctx.enter_context(tc.tile_pool(name="tpsum", bufs=2, space="PSUM"))
        mmps = ctx.enter_context(tc.tile_pool(name="mmps", bufs=3, space="PSUM"))
        stage = ctx.enter_context(tc.tile_pool(name="stage", bufs=6))

        ident = consts.tile([128, 128], FP32)
        make_identity(nc, ident)
        # Dummy transpose so PE observes the gpsimd (ident) sem here: the
        # matmul/LDW struct only carries ONE sync wait, so later transposes
        # must each need at most one sem (codegen: "Too many sync waits").
        warm = tpsum.tile([128, 128], FP32, tag="tp")
        nc.tensor.transpose(warm, ident, ident)

        # Linear chunk loads: partition p holds b rows h*1024 + 8p .. 8p+7 as
        # one contiguous 2 KB line (vs 256 B lines for the natural layout).
        b_r = b_d[:, :].rearrange("(c p j) d -> p c (j d)", c=NCH, p=128)

        def load_chunk(h):
            t = bpool.tile([128, CT, D], FP32, tag="b_nat")
            nc.sync.dma_start(
                out=t, in_=b_r[:, h, :].rearrange("p (j d) -> p j d", d=D)
            )
            return t

        # b chunk 0 in flight before the a prologue issues its DMA.
        b_tiles = {0: load_chunk(0)}

        # ---------------- a prologue ----------------
        # a loads on the ACT HW DGE queue so it transfers CONCURRENTLY with
        # b0 on the SP queue (serialized in-loads cost ~3 us of head time).
        a_nat = apool.tile([128, NT, D], FP32)        # row t*128+p at [p, t, :]
        nc.scalar.dma_start(out=a_nat, in_=a_d[:, :].rearrange("(t p) d -> p t d", p=128))
        la = apool.tile([128, NT, D], FP32)
        nc.scalar.activation(la, a_nat, AF.Ln)
        prod = apool.tile([128, NT, D], FP32)
        nc.vector.tensor_mul(prod, a_nat, la)
        ent = apool.tile([128, NT], FP32)
        for t in range(NT):
            nc.vector.reduce_sum(ent[:, t : t + 1], prod[:, t, :], axis=AX.X)

        aT = apool.tile([64, NT, 128], MMDT)          # aT[:, t, :] = a tile t transposed
        for g in range(2):
            tp = tpsum.tile([64, 4, 128], FP32, tag="tp")
            for j in range(4):
                nc.tensor.transpose(tp[:, j], a_nat[:, g * 4 + j, :], ident)
            # DVE evacuation keeps ACT free for Ln work in the head
            nc.vector.tensor_copy(aT[:, g * 4 : (g + 1) * 4, :], tp)

        # transpose raw b, fuse Ln into the PSUM->SBUF evacuation.
        # linear layout: column p of transpose j is b row h*1024+8p+j, so the
        # evac writes lbT columns {8p+j} (stride-8 view) and the GEMM still
        # reads m-contiguous 512-wide slices.
        def transpose_group(b_nat, lbT_v, g):
            tp = tpsum.tile([64, 4, 128], FP32, tag="tp")
            for j in range(4):
                nc.tensor.transpose(tp[:, j], b_nat[:, g * 4 + j, :], ident)
            nc.scalar.activation(lbT_v[:, g * 4 : (g + 1) * 4, :], tp, AF.Ln)

        def new_lbt():
            lbT = lbtp.tile([64, CT * 128], MMDT, tag="lbT")
            return lbT, lbT[:, :].rearrange("d (p j) -> d j p", j=CT)

        # chunk 0 transposes in the prologue; chunk h+1's are interleaved
        # into chunk h's n-tile loop below.
        b_tiles[1] = load_chunk(1)
        lbT_cur, v = new_lbt()
        transpose_group(b_tiles[0], v, 0)
        transpose_group(b_tiles[0], v, 1)

        # ---------------- chunk-pipelined main loop ----------------
        ev = 0
        for h in range(NCH):
            if h + 2 < NCH:
                b_tiles[h + 2] = load_chunk(h + 2)
            lbT_next = None
            for t in range(NT):
                # interleave next chunk's transposes so the evacuation
                # engines never idle at chunk boundaries
                if h + 1 < NCH and t in (2, 5):
                    if t == 2:
                        lbT_next, vn = new_lbt()
                    transpose_group(b_tiles[h + 1], vn, 0 if t == 2 else 1)
                ps = mmps.tile([128, 2, 512], FP32, tag="ps")
                for j in range(2):
                    nc.tensor.matmul(
                        ps[:, j],
                        aT[:, t, :],
                        lbT_cur[:, j * 512 : (j + 1) * 512],
                        start=True,
                        stop=True,
                    )
                out_sb = stage.tile([128, 2, 512], ODT, tag="out_sb")
                ent_t = ent[:, t : t + 1]
                # ACT also carries the Ln evacuations + la, so DVE takes a
                # slightly larger share (9 of 16).
                if ev % 16 in (0, 2, 5, 7, 9, 11, 14):
                    nc.scalar.activation(out_sb, ps, AF.Identity, bias=ent_t, scale=-1.0)
                else:
                    nc.vector.tensor_scalar(out_sb, ps, -1.0, ent_t, ALU.mult, ALU.add)
                # alternate the two HW DGE queues (SP / ACT) so per-
                # instruction queue re-arm bubbles hide behind the other
                # queue's transfers.
                dma_eng = nc.sync if ev % 2 == 0 else nc.scalar
                ev += 1
                dma_eng.dma_start(
                    out=out_d[
                        t * 128 : (t + 1) * 128, h * HCH : (h + 1) * HCH
                    ].rearrange("p (c m) -> p c m", m=512),
                    in_=out_sb,
                )
            lbT_cur = lbT_next
    # bacc lowering: splits multi-sem waits onto event-semaphore/nop
    # instructions (HW allows one sync wait per engine instruction).
    nc.compile()
    return nc


def _run(a, b, trace=False):
    from concourse.bass_utils import run_bass_kernel_spmd

    key = (MM_DTYPE, OUT_DTYPE)
    if key not in _CACHE:
        _CACHE[key] = _build(*key)
    nc = _CACHE[key]
    a = np.ascontiguousarray(np.asarray(a, dtype=np.float32))
    b = np.ascontiguousarray(np.asarray(b, dtype=np.float32))
    in_maps = [
        {"a": a[i * NSHARD : (i + 1) * NSHARD], "b": b} for i in range(NCORES)
    ]
    res = run_bass_kernel_spmd(nc, in_maps, list(range(NCORES)), trace=trace)
    out = np.concatenate(
        [np.asarray(r["out"], dtype=np.float32) for r in res.results], axis=0
    )
    return out, res


def kernel(a, b):
    out, _ = _run(a, b, trace=False)
    return out


# revision 22
# speedup vs baseline: 1.3698x; 1.0183x over previous
"""Pairwise KL divergence kernel for Trainium2, SPMD across 8 NeuronCores.

out[n, m] = sum_d a[n,d]*(log a[n,d] - log b[m,d])
          = ent[n] - (a @ log(b)^T)[n, m],  ent = rowsum(a * log a)

Sharding: a (and output rows) split 8 ways; b replicated.
Per core: a_shard (1024, 64), b (8192, 64) -> out_shard (1024, 8192).

Structure:
  - output written as bf16 (16 MB/core vs 32 MB) and converted to fp32 on
    the host; bf16 rounding adds ~2e-3 rel err, well under the 2e-2 gate.
  - b loaded in 8 linear 1024-row chunks (2 KB lines); chunk transposes on
    PE with Ln fused into the ACT PSUM->SBUF evacuation (strided write so
    the GEMM reads m-contiguous 512-wide f32r slices).
  - chunk h+1's transposes are interleaved INTO chunk h's n-tile loop so
    the evacuation engines never idle at chunk boundaries.
  - per (chunk, n-tile): 2 fp32r matmuls -> psum (128, 2, 512); evac fused
    with the entropy bias, alternating ACT/DVE 7:9; one 2 KB-line bf16 DMA
    per n-tile alternating the SP/ACT HW DGE queues (single-dependency
    DMAs only — multi-sem waits serialize the queue).
"""

import numpy as np

N, M, D = 8192, 8192, 64
NCORES = 8
NSHARD = N // NCORES          # 1024 rows of a per core
NT = NSHARD // 128            # 8 n-tiles per core
HCH = 1024                    # b rows per chunk
NCH = M // HCH                # 8 chunks
CT = HCH // 128               # 8 b row-tiles per chunk

# matmul operand dtype: "fp32" (4 cyc/row), "fp32r" (1 cyc/row), "bf16"
MM_DTYPE = "fp32r"
# output HBM dtype: "bf16" halves the output write traffic
OUT_DTYPE = "bf16"

_CACHE = {}


def _build(mm_dtype, out_dtype):
    from contextlib import ExitStack

    import concourse.bacc as bacc_mod
    import concourse.bass as bass
    import concourse.mybir as mybir
    import concourse.tile as tile
    from concourse.masks import make_identity

    FP32 = mybir.dt.float32
    AF = mybir.ActivationFunctionType
    ALU = mybir.AluOpType
    AX = mybir.AxisListType

    ODT = mybir.dt.bfloat16 if out_dtype == "bf16" else FP32
    MMDT = {
        "fp32r": mybir.dt.float32r,
        "bf16": mybir.dt.bfloat16,
        "fp32": FP32,
    }[mm_dtype]

    nc = bacc_mod.Bacc()
    a_d = nc.dram_tensor("a", [NSHARD, D], FP32, kind="ExternalInput")
    b_d = nc.dram_tensor("b", [M, D], FP32, kind="ExternalInput")
    out_d = nc.dram_tensor("out", [NSHARD, M], ODT, kind="ExternalOutput")

    with tile.TileContext(nc) as tc, ExitStack() as ctx:
        consts = ctx.enter_context(tc.tile_pool(name="consts", bufs=1))
        apool = ctx.enter_context(tc.tile_pool(name="apool", bufs=1))
        bpool = ctx.enter_context(tc.tile_pool(name="bpool", bufs=3))
        lbtp = ctx.enter_context(tc.tile_pool(name="lbtp", bufs=3))
        tpsum = ctx.enter_context(tc.tile_pool(name="tpsum", bufs=2, space="PSUM"))
        mmps = ctx.enter_context(tc.tile_pool(name="mmps", bufs=3, space="PSUM"))
        stage = ctx.enter_context(tc.tile_pool(name="stage", bufs=6))

        ident = consts.tile([128, 128], FP32)
        make_identity(nc, ident)
        # Dummy transpose so PE observes the gpsimd (ident) sem here: the
        # matmul/LDW struct only carries ONE sync wait, so later transposes
        # must each need at most one sem (codegen: "Too many sync waits").
        warm = tpsum.tile([128, 128], FP32, tag="tp")
        nc.tensor.transpose(warm, ident, ident)

        # Linear chunk loads: partition p holds b rows h*1024 + 8p .. 8p+7 as
        # one contiguous 2 KB line (vs 256 B lines for the natural layout).
        b_r = b_d[:, :].rearrange("(c p j) d -> p c (j d)", c=NCH, p=128)

        def load_chunk(h):
            t = bpool.tile([128, CT, D], FP32, tag="b_nat")
            nc.sync.dma_start(
                out=t, in_=b_r[:, h, :].rearrange("p (j d) -> p j d", d=D)
            )
            return t

        # b chunk 0 in flight before the a prologue issues its DMA.
        b_tiles = {0: load_chunk(0)}

        # ---------------- a prologue ----------------
        # a loads on the ACT HW DGE queue so it transfers CONCURRENTLY with
        # b0 on the SP queue (serialized in-loads cost ~3 us of head time).
        a_nat = apool.tile([128, NT, D], FP32)        # row t*128+p at [p, t, :]
        nc.scalar.dma_start(out=a_nat, in_=a_d[:, :].rearrange("(t p) d -> p t d", p=128))
        la = apool.tile([128, NT, D], FP32)
        nc.scalar.activation(la, a_nat, AF.Ln)
        prod = apool.tile([128, NT, D], FP32)
        nc.vector.tensor_mul(prod, a_nat, la)
        ent = apool.tile([128, NT], FP32)
        for t in range(NT):
            nc.vector.reduce_sum(ent[:, t : t + 1], prod[:, t, :], axis=AX.X)

        aT = apool.tile([64, NT, 128], MMDT)          # aT[:, t, :] = a tile t transposed
        for g in range(2):
            tp = tpsum.tile([64, 4, 128], FP32, tag="tp")
            for j in range(4):
                nc.tensor.transpose(tp[:, j], a_nat[:, g * 4 + j, :], ident)
            # DVE evacuation keeps ACT free for Ln work in the head
            nc.vector.tensor_copy(aT[:, g * 4 : (g + 1) * 4, :], tp)

        # transpose raw b, fuse Ln into the PSUM->SBUF evacuation.
        # linear layout: column p of transpose j is b row h*1024+8p+j, so the
        # evac writes lbT columns {8p+j} (stride-8 view) and the GEMM still
        # reads m-contiguous 512-wide slices.
        def transpose_group(b_nat, lbT_v, g):
            tp = tpsum.tile([64, 4, 128], FP32, tag="tp")
            for j in range(4):
                nc.tensor.transpose(tp[:, j], b_nat[:, g * 4 + j, :], ident)
            nc.scalar.activation(lbT_v[:, g * 4 : (g + 1) * 4, :], tp, AF.Ln)

        def new_lbt():
            lbT = lbtp.tile([64, CT * 128], MMDT, tag="lbT")
            return lbT, lbT[:, :].rearrange("d (p j) -> d j p", j=CT)

        # chunks 0+1 transpose in the prologue (two-chunk lookahead); chunk
        # h+2's transposes are interleaved into chunk h's n-tile loop below.
        b_tiles[1] = load_chunk(1)
        b_tiles[2] = load_chunk(2)
        lbT_cur, v = new_lbt()
        transpose_group(b_tiles[0], v, 0)
        transpose_group(b_tiles[0], v, 1)
        lbT_next, vn = new_lbt()
        transpose_group(b_tiles[1], vn, 0)
        transpose_group(b_tiles[1], vn, 1)

        # ---------------- chunk-pipelined main loop ----------------
        ev = 0
        for h in range(NCH):
            if h + 3 < NCH:
                b_tiles[h + 3] = load_chunk(h + 3)
            lbT_next2 = None
            for t in range(NT):
                # interleave chunk h+2's transposes so the evacuation
                # engines never idle at chunk boundaries
                if h + 2 < NCH and t in (2, 5):
                    if t == 2:
                        lbT_next2, vn2 = new_lbt()
                    transpose_group(b_tiles[h + 2], vn2, 0 if t == 2 else 1)
                ps = mmps.tile([128, 2, 512], FP32, tag="ps")
                for j in range(2):
                    nc.tensor.matmul(
                        ps[:, j],
                        aT[:, t, :],
                        lbT_cur[:, j * 512 : (j + 1) * 512],
                        start=True,
                        stop=True,
                    )
                out_sb = stage.tile([128, 2, 512], ODT, tag="out_sb")
                ent_t = ent[:, t : t + 1]
                # ACT also carries the Ln evacuations + la, so DVE takes a
                # slightly larger share (9 of 16).
                if ev % 16 in (0, 2, 5, 9, 11, 14):
                    nc.scalar.activation(out_sb, ps, AF.Identity, bias=ent_t, scale=-1.0)
                else:
                    nc.vector.tensor_scalar(out_sb, ps, -1.0, ent_t, ALU.mult, ALU.add)
                # alternate the two HW DGE queues (SP / ACT) so per-
                # instruction queue re-arm bubbles hide behind the other
                # queue's transfers.
                dma_eng = nc.sync if ev % 2 == 0 else nc.scalar
                ev += 1
                dma_eng.dma_start(
                    out=out_d[
                        t * 128 : (t + 1) * 128, h * HCH : (h + 1) * HCH
                    ].rearrange("p (c m) -> p c m", m=512),
                    in_=out_sb,
                )
            lbT_cur, lbT_next = lbT_next, lbT_next2
    # bacc lowering: splits multi-sem waits onto event-semaphore/nop
    # instructions (HW allows one sync wait per engine instruction).
    nc.compile()
    return nc


def _run(a, b, trace=False):
    from concourse.bass_utils import run_bass_kernel_spmd

    key = (MM_DTYPE, OUT_DTYPE)
    if key not in _CACHE:
        _CACHE[key] = _build(*key)
    nc = _CACHE[key]
    a = np.ascontiguousarray(np.asarray(a, dtype=np.float32))
    b = np.ascontiguousarray(np.asarray(b, dtype=np.float32))
    in_maps = [
        {"a": a[i * NSHARD : (i + 1) * NSHARD], "b": b} for i in range(NCORES)
    ]
    res = run_bass_kernel_spmd(nc, in_maps, list(range(NCORES)), trace=trace)
    out = np.concatenate(
        [np.asarray(r["out"], dtype=np.float32) for r in res.results], axis=0
    )
    return out, res


def kernel(a, b):
    out, _ = _run(a, b, trace=False)
    return out


# revision 23
# speedup vs baseline: 1.3759x; 1.0045x over previous
"""Pairwise KL divergence kernel for Trainium2, SPMD across 8 NeuronCores.

out[n, m] = sum_d a[n,d]*(log a[n,d] - log b[m,d])
          = ent[n] - (a @ log(b)^T)[n, m],  ent = rowsum(a * log a)

Sharding: a (and output rows) split 8 ways; b replicated.
Per core: a_shard (1024, 64), b (8192, 64) -> out_shard (1024, 8192).

Structure:
  - output written as bf16 (16 MB/core vs 32 MB) and converted to fp32 on
    the host; bf16 rounding adds ~2e-3 rel err, well under the 2e-2 gate.
  - b loaded in 8 linear 1024-row chunks (2 KB lines); chunk transposes on
    PE with Ln fused into the ACT PSUM->SBUF evacuation (strided write so
    the GEMM reads m-contiguous 512-wide f32r slices).
  - chunk h+1's transposes are interleaved INTO chunk h's n-tile loop so
    the evacuation engines never idle at chunk boundaries.
  - per (chunk, n-tile): 2 fp32r matmuls -> psum (128, 2, 512); evac fused
    with the entropy bias, alternating ACT/DVE 7:9; one 2 KB-line bf16 DMA
    per n-tile alternating the SP/ACT HW DGE queues (single-dependency
    DMAs only — multi-sem waits serialize the queue).
"""

import numpy as np

N, M, D = 8192, 8192, 64
NCORES = 8
NSHARD = N // NCORES          # 1024 rows of a per core
NT = NSHARD // 128            # 8 n-tiles per core
HCH = 1024                    # b rows per chunk
NCH = M // HCH                # 8 chunks
CT = HCH // 128               # 8 b row-tiles per chunk

# matmul operand dtype: "fp32" (4 cyc/row), "fp32r" (1 cyc/row), "bf16"
MM_DTYPE = "fp32r"
# output HBM dtype: 2-byte dtypes halve the output write traffic; fp16 has
# 10 mantissa bits (vs bf16's 7) and |out| <= ~450 fits fp16 range easily
OUT_DTYPE = "fp16"

_CACHE = {}


def _build(mm_dtype, out_dtype):
    from contextlib import ExitStack

    import concourse.bacc as bacc_mod
    import concourse.bass as bass
    import concourse.mybir as mybir
    import concourse.tile as tile
    from concourse.masks import make_identity

    FP32 = mybir.dt.float32
    AF = mybir.ActivationFunctionType
    ALU = mybir.AluOpType
    AX = mybir.AxisListType

    ODT = {
        "bf16": mybir.dt.bfloat16,
        "fp16": mybir.dt.float16,
        "fp32": FP32,
    }[out_dtype]
    MMDT = {
        "fp32r": mybir.dt.float32r,
        "bf16": mybir.dt.bfloat16,
        "fp32": FP32,
    }[mm_dtype]

    nc = bacc_mod.Bacc()
    a_d = nc.dram_tensor("a", [NSHARD, D], FP32, kind="ExternalInput")
    b_d = nc.dram_tensor("b", [M, D], FP32, kind="ExternalInput")
    out_d = nc.dram_tensor("out", [NSHARD, M], ODT, kind="ExternalOutput")

    with tile.TileContext(nc) as tc, ExitStack() as ctx:
        consts = ctx.enter_context(tc.tile_pool(name="consts", bufs=1))
        apool = ctx.enter_context(tc.tile_pool(name="apool", bufs=1))
        bpool = ctx.enter_context(tc.tile_pool(name="bpool", bufs=3))
        lbtp = ctx.enter_context(tc.tile_pool(name="lbtp", bufs=3))
        tpsum = ctx.enter_context(tc.tile_pool(name="tpsum", bufs=2, space="PSUM"))
        mmps = ctx.enter_context(tc.tile_pool(name="mmps", bufs=3, space="PSUM"))
        stage = ctx.enter_context(tc.tile_pool(name="stage", bufs=10))

        ident = consts.tile([128, 128], FP32)
        make_identity(nc, ident)
        # Dummy transpose so PE observes the gpsimd (ident) sem here: the
        # matmul/LDW struct only carries ONE sync wait, so later transposes
        # must each need at most one sem (codegen: "Too many sync waits").
        warm = tpsum.tile([128, 128], FP32, tag="tp")
        nc.tensor.transpose(warm, ident, ident)

        # Linear chunk loads: partition p holds b rows h*1024 + 8p .. 8p+7 as
        # one contiguous 2 KB line (vs 256 B lines for the natural layout).
        b_r = b_d[:, :].rearrange("(c p j) d -> p c (j d)", c=NCH, p=128)

        def load_chunk(h):
            t = bpool.tile([128, CT, D], FP32, tag="b_nat")
            nc.sync.dma_start(
                out=t, in_=b_r[:, h, :].rearrange("p (j d) -> p j d", d=D)
            )
            return t

        # b chunk 0 in flight before the a prologue issues its DMA.
        b_tiles = {0: load_chunk(0)}

        # ---------------- a prologue ----------------
        # a loads on the ACT HW DGE queue so it transfers CONCURRENTLY with
        # b0 on the SP queue (serialized in-loads cost ~3 us of head time).
        a_nat = apool.tile([128, NT, D], FP32)        # row t*128+p at [p, t, :]
        nc.scalar.dma_start(out=a_nat, in_=a_d[:, :].rearrange("(t p) d -> p t d", p=128))
        la = apool.tile([128, NT, D], FP32)
        nc.scalar.activation(la, a_nat, AF.Ln)
        prod = apool.tile([128, NT, D], FP32)
        nc.vector.tensor_mul(prod, a_nat, la)
        ent = apool.tile([128, NT], FP32)
        for t in range(NT):
            nc.vector.reduce_sum(ent[:, t : t + 1], prod[:, t, :], axis=AX.X)

        aT = apool.tile([64, NT, 128], MMDT)          # aT[:, t, :] = a tile t transposed
        for g in range(2):
            tp = tpsum.tile([64, 4, 128], FP32, tag="tp")
            for j in range(4):
                nc.tensor.transpose(tp[:, j], a_nat[:, g * 4 + j, :], ident)
            # DVE evacuation keeps ACT free for Ln work in the head
            nc.vector.tensor_copy(aT[:, g * 4 : (g + 1) * 4, :], tp)

        # transpose raw b, fuse Ln into the PSUM->SBUF evacuation.
        # linear layout: column p of transpose j is b row h*1024+8p+j, so the
        # evac writes lbT columns {8p+j} (stride-8 view) and the GEMM still
        # reads m-contiguous 512-wide slices.
        def transpose_group(b_nat, lbT_v, g):
            tp = tpsum.tile([64, 4, 128], FP32, tag="tp")
            for j in range(4):
                nc.tensor.transpose(tp[:, j], b_nat[:, g * 4 + j, :], ident)
            nc.scalar.activation(lbT_v[:, g * 4 : (g + 1) * 4, :], tp, AF.Ln)

        def new_lbt():
            lbT = lbtp.tile([64, CT * 128], MMDT, tag="lbT")
            return lbT, lbT[:, :].rearrange("d (p j) -> d j p", j=CT)

        # chunks 0+1 transpose in the prologue (two-chunk lookahead); chunk
        # h+2's transposes are interleaved into chunk h's n-tile loop below.
        b_tiles[1] = load_chunk(1)
        b_tiles[2] = load_chunk(2)
        lbT_cur, v = new_lbt()
        transpose_group(b_tiles[0], v, 0)
        transpose_group(b_tiles[0], v, 1)
        lbT_next, vn = new_lbt()
        transpose_group(b_tiles[1], vn, 0)
        transpose_group(b_tiles[1], vn, 1)

        # ---------------- chunk-pipelined main loop ----------------
        ev = 0
        for h in range(NCH):
            if h + 3 < NCH:
                b_tiles[h + 3] = load_chunk(h + 3)
            lbT_next2 = None
            for t in range(NT):
                # interleave chunk h+2's transposes so the evacuation
                # engines never idle at chunk boundaries
                if h + 2 < NCH and t in (2, 5):
                    if t == 2:
                        lbT_next2, vn2 = new_lbt()
                    transpose_group(b_tiles[h + 2], vn2, 0 if t == 2 else 1)
                ps = mmps.tile([128, 2, 512], FP32, tag="ps")
                for j in range(2):
                    nc.tensor.matmul(
                        ps[:, j],
                        aT[:, t, :],
                        lbT_cur[:, j * 512 : (j + 1) * 512],
                        start=True,
                        stop=True,
                    )
                out_sb = stage.tile([128, 2, 512], ODT, tag="out_sb")
                ent_t = ent[:, t : t + 1]
                # ACT also carries the Ln evacuations + la, so DVE takes a
                # slightly larger share (9 of 16).
                if ev % 16 in (0, 2, 5, 9, 11, 14):
                    nc.scalar.activation(out_sb, ps, AF.Identity, bias=ent_t, scale=-1.0)
                else:
                    nc.vector.tensor_scalar(out_sb, ps, -1.0, ent_t, ALU.mult, ALU.add)
                # alternate the two HW DGE queues (SP / ACT) so per-
                # instruction queue re-arm bubbles hide behind the other
                # queue's transfers.
                dma_eng = nc.sync if ev % 2 == 0 else nc.scalar
                ev += 1
                dma_eng.dma_start(
                    out=out_d[
                        t * 128 : (t + 1) * 128, h * HCH : (h + 1) * HCH
                    ].rearrange("p (c m) -> p c m", m=512),
                    in_=out_sb,
                )
            lbT_cur, lbT_next = lbT_next, lbT_next2
    # bacc lowering: splits multi-sem waits onto event-semaphore/nop
    # instructions (HW allows one sync wait per engine instruction).
    nc.compile()
    return nc


def _run(a, b, trace=False):
    from concourse.bass_utils import run_bass_kernel_spmd

    key = (MM_DTYPE, OUT_DTYPE)
    if key not in _CACHE:
        _CACHE[key] = _build(*key)
    nc = _CACHE[key]
    a = np.ascontiguousarray(np.asarray(a, dtype=np.float32))
    b = np.ascontiguousarray(np.asarray(b, dtype=np.float32))
    in_maps = [
        {"a": a[i * NSHARD : (i + 1) * NSHARD], "b": b} for i in range(NCORES)
    ]
    res = run_bass_kernel_spmd(nc, in_maps, list(range(NCORES)), trace=trace)
    out = np.concatenate(
        [np.asarray(r["out"], dtype=np.float32) for r in res.results], axis=0
    )
    return out, res


def kernel(a, b):
    out, _ = _run(a, b, trace=False)
    return out


# revision 24
# speedup vs baseline: 1.3926x; 1.0121x over previous
"""Pairwise KL divergence kernel for Trainium2, SPMD across 8 NeuronCores.

out[n, m] = sum_d a[n,d]*(log a[n,d] - log b[m,d])
          = ent[n] - (a @ log(b)^T)[n, m],  ent = rowsum(a * log a)

Sharding: a (and output rows) split 8 ways; b replicated.
Per core: a_shard (1024, 64), b (8192, 64) -> out_shard (1024, 8192).

Structure:
  - output written as fp16 (16 MB/core vs 32 MB fp32) and converted to
    fp32 on the host; fp16 rounding adds ~4e-4 rel err vs the 2e-2 gate,
    and |out| <= ~450 fits fp16 range easily.
  - b loaded in 8 linear 1024-row chunks (2 KB lines, vs 256 B lines for
    the natural layout); chunk transposes on PE with Ln fused into the
    ACT PSUM->SBUF evacuation (stride-8 write so the GEMM still reads
    m-contiguous 512-wide f32r slices).
  - two-chunk lookahead: chunks 0+1 transpose in the prologue and chunk
    h+2's transposes are interleaved INTO chunk h's n-tile loop, so the
    evacuation engines never idle at chunk boundaries.
  - per (chunk, n-tile): 2 fp32r matmuls -> psum (128, 2, 512); evac
    fused with the entropy bias, split ACT:DVE = 6:10 (ACT also carries
    the Ln evacuations); one 2 KB-line fp16 DMA per n-tile alternating
    the SP/ACT HW DGE queues. All DMAs single-dependency — multi-sem
    waits lower to event-semaphore shims that serialize the queue.
  - in-loads run concurrently: b chunks on the SP queue, a on the ACT
    queue; a's ent chain (Ln -> mul -> rowsum) overlaps the b0 pipeline.
"""

import numpy as np

N, M, D = 8192, 8192, 64
NCORES = 8
NSHARD = N // NCORES          # 1024 rows of a per core
NT = NSHARD // 128            # 8 n-tiles per core
HCH = 1024                    # b rows per chunk
NCH = M // HCH                # 8 chunks
CT = HCH // 128               # 8 b row-tiles per chunk

# matmul operand dtype: "fp32" (4 cyc/row), "fp32r" (1 cyc/row), "bf16"
MM_DTYPE = "fp32r"
# output HBM dtype: 2-byte dtypes halve the output write traffic; fp16 has
# 10 mantissa bits (vs bf16's 7) and |out| <= ~450 fits fp16 range easily
OUT_DTYPE = "fp16"

_CACHE = {}


def _build(mm_dtype, out_dtype):
    from contextlib import ExitStack

    import concourse.bacc as bacc_mod
    import concourse.bass as bass
    import concourse.mybir as mybir
    import concourse.tile as tile
    from concourse.masks import make_identity

    FP32 = mybir.dt.float32
    AF = mybir.ActivationFunctionType
    ALU = mybir.AluOpType
    AX = mybir.AxisListType

    ODT = {
        "bf16": mybir.dt.bfloat16,
        "fp16": mybir.dt.float16,
        "fp32": FP32,
    }[out_dtype]
    MMDT = {
        "fp32r": mybir.dt.float32r,
        "bf16": mybir.dt.bfloat16,
        "fp32": FP32,
    }[mm_dtype]

    nc = bacc_mod.Bacc()
    a_d = nc.dram_tensor("a", [NSHARD, D], FP32, kind="ExternalInput")
    b_d = nc.dram_tensor("b", [M, D], FP32, kind="ExternalInput")
    out_d = nc.dram_tensor("out", [NSHARD, M], ODT, kind="ExternalOutput")

    with tile.TileContext(nc) as tc, ExitStack() as ctx:
        consts = ctx.enter_context(tc.tile_pool(name="consts", bufs=1))
        apool = ctx.enter_context(tc.tile_pool(name="apool", bufs=1))
        bpool = ctx.enter_context(tc.tile_pool(name="bpool", bufs=3))
        lbtp = ctx.enter_context(tc.tile_pool(name="lbtp", bufs=3))
        tpsum = ctx.enter_context(tc.tile_pool(name="tpsum", bufs=2, space="PSUM"))
        mmps = ctx.enter_context(tc.tile_pool(name="mmps", bufs=3, space="PSUM"))
        stage = ctx.enter_context(tc.tile_pool(name="stage", bufs=10))

        ident = consts.tile([128, 128], FP32)
        make_identity(nc, ident)
        # Dummy transpose so PE observes the gpsimd (ident) sem here: the
        # matmul/LDW struct only carries ONE sync wait, so later transposes
        # must each need at most one sem (codegen: "Too many sync waits").
        warm = tpsum.tile([128, 128], FP32, tag="tp")
        nc.tensor.transpose(warm, ident, ident)

        # Linear chunk loads: partition p holds b rows h*1024 + 8p .. 8p+7 as
        # one contiguous 2 KB line (vs 256 B lines for the natural layout).
        b_r = b_d[:, :].rearrange("(c p j) d -> p c (j d)", c=NCH, p=128)

        def load_chunk(h):
            t = bpool.tile([128, CT, D], FP32, tag="b_nat")
            nc.sync.dma_start(
                out=t, in_=b_r[:, h, :].rearrange("p (j d) -> p j d", d=D)
            )
            return t

        # b chunk 0 in flight before the a prologue issues its DMA.
        b_tiles = {0: load_chunk(0)}

        # ---------------- a prologue ----------------
        # a loads on the ACT HW DGE queue so it transfers CONCURRENTLY with
        # b0 on the SP queue (serialized in-loads cost ~3 us of head time).
        a_nat = apool.tile([128, NT, D], FP32)        # row t*128+p at [p, t, :]
        nc.scalar.dma_start(out=a_nat, in_=a_d[:, :].rearrange("(t p) d -> p t d", p=128))
        la = apool.tile([128, NT, D], FP32)
        nc.scalar.activation(la, a_nat, AF.Ln)
        prod = apool.tile([128, NT, D], FP32)
        nc.vector.tensor_mul(prod, a_nat, la)
        ent = apool.tile([128, NT], FP32)
        for t in range(NT):
            nc.vector.reduce_sum(ent[:, t : t + 1], prod[:, t, :], axis=AX.X)

        aT = apool.tile([64, NT, 128], MMDT)          # aT[:, t, :] = a tile t transposed
        for g in range(2):
            tp = tpsum.tile([64, 4, 128], FP32, tag="tp")
            for j in range(4):
                nc.tensor.transpose(tp[:, j], a_nat[:, g * 4 + j, :], ident)
            # DVE evacuation keeps ACT free for Ln work in the head
            nc.vector.tensor_copy(aT[:, g * 4 : (g + 1) * 4, :], tp)

        # transpose raw b, fuse Ln into the PSUM->SBUF evacuation.
        # linear layout: column p of transpose j is b row h*1024+8p+j, so the
        # evac writes lbT columns {8p+j} (stride-8 view) and the GEMM still
        # reads m-contiguous 512-wide slices.
        def transpose_group(b_nat, lbT_v, g):
            tp = tpsum.tile([64, 4, 128], FP32, tag="tp")
            for j in range(4):
                nc.tensor.transpose(tp[:, j], b_nat[:, g * 4 + j, :], ident)
            nc.scalar.activation(lbT_v[:, g * 4 : (g + 1) * 4, :], tp, AF.Ln)

        def new_lbt():
            lbT = lbtp.tile([64, CT * 128], MMDT, tag="lbT")
            return lbT, lbT[:, :].rearrange("d (p j) -> d j p", j=CT)

        # chunks 0+1 transpose in the prologue (two-chunk lookahead); chunk
        # h+2's transposes are interleaved into chunk h's n-tile loop below.
        b_tiles[1] = load_chunk(1)
        b_tiles[2] = load_chunk(2)
        lbT_cur, v = new_lbt()
        transpose_group(b_tiles[0], v, 0)
        transpose_group(b_tiles[0], v, 1)
        lbT_next, vn = new_lbt()
        transpose_group(b_tiles[1], vn, 0)
        transpose_group(b_tiles[1], vn, 1)

        # ---------------- chunk-pipelined main loop ----------------
        ev = 0
        for h in range(NCH):
            if h + 3 < NCH:
                b_tiles[h + 3] = load_chunk(h + 3)
            lbT_next2 = None
            for t in range(NT):
                # interleave chunk h+2's transposes so the evacuation
                # engines never idle at chunk boundaries
                if h + 2 < NCH and t in (2, 5):
                    if t == 2:
                        lbT_next2, vn2 = new_lbt()
                    transpose_group(b_tiles[h + 2], vn2, 0 if t == 2 else 1)
                ps = mmps.tile([128, 2, 512], FP32, tag="ps")
                for j in range(2):
                    nc.tensor.matmul(
                        ps[:, j],
                        aT[:, t, :],
                        lbT_cur[:, j * 512 : (j + 1) * 512],
                        start=True,
                        stop=True,
                    )
                out_sb = stage.tile([128, 2, 512], ODT, tag="out_sb")
                ent_t = ent[:, t : t + 1]
                # ACT also carries the Ln evacuations + la, so DVE takes a
                # slightly larger share (9 of 16).
                if ev % 16 in (0, 2, 5, 9, 11, 14):
                    nc.scalar.activation(out_sb, ps, AF.Identity, bias=ent_t, scale=-1.0)
                else:
                    nc.vector.tensor_scalar(out_sb, ps, -1.0, ent_t, ALU.mult, ALU.add)
                # alternate the two HW DGE queues (SP / ACT) so per-
                # instruction queue re-arm bubbles hide behind the other
                # queue's transfers.
                dma_eng = nc.sync if ev % 2 == 0 else nc.scalar
                ev += 1
                dma_eng.dma_start(
                    out=out_d[
                        t * 128 : (t + 1) * 128, h * HCH : (h + 1) * HCH
                    ].rearrange("p (c m) -> p c m", m=512),
                    in_=out_sb,
                )
            lbT_cur, lbT_next = lbT_next, lbT_next2
    # bacc lowering: splits multi-sem waits onto event-semaphore/nop
    # instructions (HW allows one sync wait per engine instruction).
    nc.compile()
    return nc


def _run(a, b, trace=False):
    from concourse.bass_utils import run_bass_kernel_spmd

    key = (MM_DTYPE, OUT_DTYPE)
    if key not in _CACHE:
        _CACHE[key] = _build(*key)
    nc = _CACHE[key]
    a = np.ascontiguousarray(np.asarray(a, dtype=np.float32))
    b = np.ascontiguousarray(np.asarray(b, dtype=np.float32))
    in_maps = [
        {"a": a[i * NSHARD : (i + 1) * NSHARD], "b": b} for i in range(NCORES)
    ]
    res = run_bass_kernel_spmd(nc, in_maps, list(range(NCORES)), trace=trace)
    out = np.concatenate(
        [np.asarray(r["out"], dtype=np.float32) for r in res.results], axis=0
    )
    return out, res


def kernel(a, b):
    out, _ = _run(a, b, trace=False)
    return out
